# revision 27
# baseline (speedup 1.0000x reference)
"""Multi-head self-attention (B=4, L=2048, D=1024, H=16) on 8 trn2 cores.

Sharding: DP=4 over batch x TP=2 over heads (8 heads/core). Each core:
  QT/KT = W.T @ x.T projections (c on partitions), V natural layout,
  S^T = K Q^T per head with keys on partitions, exp on ACT, causal via
  tile skipping + gpsimd affine_select zeroing of P on diagonal tiles,
  ctx^T = V'.T @ P^T with a ones-column in V' producing the softmax
  denominator row, normalization via fast-approx reciprocal (DVE) +
  gpsimd partition_broadcast + DVE multiply, then the output projection
  produces a per-core partial [L, D] in f16. Host sums the TP pair
  partials + b_lin in f32.

Phases are interleaved per 512-query block (proj -> attn -> prev
block's outproj) to keep the PE busy continuously (HAM stays warm).
"""

import sys

if "/opt/trn_rl_repo" not in sys.path:
    sys.path.insert(0, "/opt/trn_rl_repo")

import numpy as np

import concourse.bass as bass
import concourse.tile as tile
from concourse import bacc, mybir
from concourse.bass_utils import run_bass_kernel_spmd

B, L, D = 4, 2048, 1024
H, HD = 16, 64
HPC = H // 2          # heads per core (TP=2)
CPC = HPC * HD        # head-dim columns per core = 512
N_CORES = 8

F32 = mybir.dt.float32
F16 = mybir.dt.float16

NB = 4                # 512-wide i/l blocks
BW = L // NB          # 512
NT = L // 128         # 16 j/l tiles of 128
DT = D // 128         # 8 d-tiles
CT = CPC // 128       # 4 c-tiles per core


def build(tc, io):
    nc = tc.nc
    # host pre-packs weights/x into SBUF layout: DMA lines are 4-32KB
    xbT = io["xbT"]       # [128, 8, 2048] f16 (packed d-tiles)
    wq = io["wq"]         # [128, 8, 512] f16
    wk = io["wk"]
    wv = io["wv"]
    wo = io["wo"]         # [128, 4, 1024] f16
    bq = io["bq"].rearrange("(t p) -> p t", p=128)            # [128,4] f32
    bk = io["bk"].rearrange("(t p) -> p t", p=128)
    bv = io["bv"].rearrange("(t p) -> p t", p=128)
    out = io["out"].rearrange("(t p) e -> t p e", p=128)      # [16,128,1024] f16

    singles = tc.alloc_tile_pool(name="singles", bufs=1)
    work = tc.alloc_tile_pool(name="work", bufs=4)
    psum = tc.alloc_tile_pool(name="psum", bufs=2, space="PSUM")

    # --- resident tensors (packed DMAs: 4-8KB contiguous per partition) -----
    wq_sb = singles.tile([128, DT, CPC], F16)
    wk_sb = singles.tile([128, DT, CPC], F16)
    wv_sb = singles.tile([128, DT, CPC], F16)
    wo_sb = singles.tile([128, CT, D], F16)
    x_sb = singles.tile([128, DT, L], F16)
    nc.sync.dma_start(out=wq_sb, in_=wq)
    # per-dt x loads so the first Q matmuls start after ~2 DMAs; wk/wv
    # interleave into the stream so K/V projections never wait on them
    for dt in range(2):
        nc.sync.dma_start(out=x_sb[:, dt], in_=xbT[:, dt])
    nc.sync.dma_start(out=wk_sb, in_=wk)
    for dt in range(2, 4):
        nc.sync.dma_start(out=x_sb[:, dt], in_=xbT[:, dt])
    nc.sync.dma_start(out=wv_sb, in_=wv)
    for dt in range(4, DT):
        nc.sync.dma_start(out=x_sb[:, dt], in_=xbT[:, dt])
    nc.sync.dma_start(out=wo_sb, in_=wo)
    bq_sb = singles.tile([128, CT], F32)
    bk_sb = singles.tile([128, CT], F32)
    bv_sb = singles.tile([128, CT], F32)
    nc.sync.dma_start(out=bq_sb, in_=bq)
    nc.sync.dma_start(out=bk_sb, in_=bk)
    nc.sync.dma_start(out=bv_sb, in_=bv)

    qt_sb = singles.tile([128, CT, L], F16)    # Q^T  [c, i]
    kt_sb = singles.tile([128, CT, L], F16)    # K^T  [c, j]
    vp_sb = singles.tile([128, NT, HPC, HD + 1], F16)   # V' [j, (h, d'|1)]
    cxt_sb = singles.tile([128, CT, L], F16)   # normalized ctx^T [d', i]
    nc.gpsimd.memset(vp_sb[:, :, :, HD : HD + 1], 1.0)
    # 1 where free-idx >= partition-idx (kept region of a diagonal tile)
    mask2_sb = singles.tile([128, 2, BW], F16)
    nc.gpsimd.memset(mask2_sb, 1.0)
    nc.gpsimd.affine_select(
        out=mask2_sb, in_=mask2_sb, compare_op=mybir.AluOpType.is_ge,
        fill=0.0, base=0, pattern=[[0, 2], [1, BW]], channel_multiplier=-1)

    def proj(blk):
        xs = x_sb[:, :, blk * BW : (blk + 1) * BW]   # [128, DT, BW]
        for w_sb, b_sb, t_sb, nm in ((wq_sb, bq_sb, qt_sb, "q"),
                                     (wk_sb, bk_sb, kt_sb, "k")):
            for cp in range(2):
                ps_p = [psum.tile([128, BW], F32, tag="pp", bufs=2,
                                  name=f"ps{nm}{blk}{cp}_{ci}") for ci in range(2)]
                for dt in range(DT):
                    for ci in range(2):
                        nc.tensor.matmul(
                            ps_p[ci],
                            lhsT=w_sb[:, dt, (2 * cp + ci) * 128 : (2 * cp + ci + 1) * 128],
                            rhs=xs[:, dt], start=(dt == 0), stop=(dt == DT - 1))
                for ci in range(2):
                    ct = 2 * cp + ci
                    nc.vector.tensor_scalar_add(
                        out=t_sb[:, ct, blk * BW : (blk + 1) * BW],
                        in0=ps_p[ci], scalar1=b_sb[:, ct : ct + 1])
        for j in range(4):
            lt = 4 * blk + j
            ps_v = psum.tile([128, CPC], F32, tag="pp", bufs=2,
                             name=f"psv{lt}")
            for dt in range(DT):
                nc.tensor.matmul(
                    ps_v, lhsT=xs[:, dt, j * 128 : (j + 1) * 128],
                    rhs=wv_sb[:, dt, :], start=(dt == 0), stop=(dt == DT - 1))
            # V bias is folded in after normalization (bv * sum(P) = bv).
            nc.vector.tensor_copy(
                out=vp_sb[:, lt, :, 0:HD],
                in_=ps_v.rearrange("p (h d) -> p h d", h=HPC))

    def attn(hp, ib):
        ctx_ps = [psum.tile([128, BW], F32, tag="ctx", bufs=2,
                            name=f"cx{hp}_{ib}_{hh}") for hh in range(2)]
        ntj = 4 * ib + 4
        for tj in range(ntj):
            k = tj - 4 * ib
            off = 128 * k if k >= 0 else 0
            w = BW - off
            st = psum.tile([128, 2, BW], F32, tag="big", bufs=2,
                           name=f"st{hp}_{ib}_{tj}")
            pt = work.tile([128, 2, BW], F16, tag="pt", name=f"pt{hp}_{ib}_{tj}")
            for hh in range(2):
                nc.tensor.matmul(
                    st[:, hh, off:], skip_group_check=True, start=True, stop=True,
                    lhsT=kt_sb[64 * hh : 64 * hh + 64, hp,
                               tj * 128 : (tj + 1) * 128],
                    rhs=qt_sb[64 * hh : 64 * hh + 64, hp,
                              ib * BW + off : (ib + 1) * BW])
            nc.scalar.activation(
                out=pt[:, :, off:], in_=st[:, :, off:],
                func=mybir.ActivationFunctionType.Exp, scale=0.125)
            if k >= 0:
                # zero the j>i triangle of this diagonal tile (query-local
                # index f vs key-local index p share the same off origin)
                nc.vector.tensor_tensor(
                    out=pt[:, :, off:], in0=pt[:, :, off:],
                    in1=mask2_sb[:, :, 0:w], op=mybir.AluOpType.mult)
            for hh in range(2):
                nc.tensor.matmul(
                    ctx_ps[hh][0 : HD + 1, off:], skip_group_check=True,
                    lhsT=vp_sb[:, tj, 2 * hp + hh, :],
                    rhs=pt[:, hh, off:], start=(tj == 0), stop=(tj == ntj - 1))
        for hh in range(2):
            # contiguous partition-0 tiles only: the custom DVE/gpsimd ops
            # drop input partition offsets on HW, so stage the denominator
            # row at partition 0 with a standard copy first.
            dcp = work.tile([1, BW], F32, tag="dcp", name=f"dc{hp}{ib}{hh}")
            rden = work.tile([1, BW], F32, tag="rden", name=f"rd{hp}{ib}{hh}")
            bcst = work.tile([64, BW], F32, tag="bcs", name=f"bs{hp}{ib}{hh}")
            nc.vector.tensor_copy(out=dcp, in_=ctx_ps[hh][HD : HD + 1, :])
            nc.vector.reciprocal_approx_fast(out=rden, in_=dcp)
            nc.gpsimd.partition_broadcast(out_ap=bcst, in_ap=rden)
            dst = cxt_sb[64 * hh : 64 * hh + 64, hp, ib * BW : (ib + 1) * BW]
            nc.vector.tensor_tensor(
                out=dst, in0=ctx_ps[hh][0:HD, :], in1=bcst,
                op=mybir.AluOpType.mult)
            nc.vector.tensor_scalar_add(
                out=dst, in0=dst,
                scalar1=bv_sb[64 * hh : 64 * hh + 64, hp : hp + 1])

    def outproj(it):
        ps_o = [psum.tile([128, 512], F32, tag="pp", bufs=2,
                          name=f"po{it}_{eb}") for eb in range(2)]
        for ct in range(CT):
            for eb in range(2):
                nc.tensor.matmul(
                    ps_o[eb],
                    lhsT=cxt_sb[:, ct, it * 128 : (it + 1) * 128],
                    rhs=wo_sb[:, ct, eb * 512 : (eb + 1) * 512],
                    start=(ct == 0), stop=(ct == CT - 1))
        o_sb = work.tile([128, D], F16, tag="osb", name=f"os{it}")
        for eb in range(2):
            nc.vector.tensor_copy(out=o_sb[:, eb * 512 : (eb + 1) * 512],
                                  in_=ps_o[eb])
        nc.sync.dma_start(out=out[it], in_=o_sb)

    # software pipeline: outproj trails attn by one block and is issued
    # BEFORE the next attn so the in-order PE queue front-loads ready work
    # (attn's exp-gated matmuls are the stall-prone part).
    for blk in range(NB):
        proj(blk)
        if blk > 0:
            for j in range(4):
                outproj(4 * (blk - 1) + j)
        for hp in range(CT):
            attn(hp, blk)
    for j in range(4):
        outproj(4 * (NB - 1) + j)

    for p in (psum, work, singles):
        p.release()


_CACHE = {}


def _compiled():
    if "nc" in _CACHE:
        return _CACHE["nc"]
    nc = bacc.Bacc("TRN2", target_bir_lowering=False, debug=False)
    io = {
        "xbT": nc.dram_tensor("xbT", [128, DT, L], F16, kind="ExternalInput").ap(),
        "wq": nc.dram_tensor("wq", [128, DT, CPC], F16, kind="ExternalInput").ap(),
        "wk": nc.dram_tensor("wk", [128, DT, CPC], F16, kind="ExternalInput").ap(),
        "wv": nc.dram_tensor("wv", [128, DT, CPC], F16, kind="ExternalInput").ap(),
        "wo": nc.dram_tensor("wo", [128, CT, D], F16, kind="ExternalInput").ap(),
        "bq": nc.dram_tensor("bq", [CPC], F32, kind="ExternalInput").ap(),
        "bk": nc.dram_tensor("bk", [CPC], F32, kind="ExternalInput").ap(),
        "bv": nc.dram_tensor("bv", [CPC], F32, kind="ExternalInput").ap(),
        "out": nc.dram_tensor("out", [L, D], F16, kind="ExternalOutput").ap(),
    }
    with tile.TileContext(nc) as tc:
        build(tc, io)
    nc.compile()
    _CACHE["nc"] = nc
    return nc


def _pack(a, p=128):
    """[T*p, N] -> [p, T, N] (SBUF layout: partition-major)."""
    t = a.shape[0] // p
    return np.ascontiguousarray(
        a.reshape(t, p, a.shape[1]).transpose(1, 0, 2))


def make_in_maps(x, W_qkv, b_qkv, W_lin):
    f16 = mybir.dt.np(F16)
    in_maps = []
    for c in range(N_CORES):
        b, g = divmod(c, 2)
        cs = slice(CPC * g, CPC * (g + 1))
        in_maps.append({
            "xbT": _pack(np.ascontiguousarray(x[b].T).astype(f16)),
            "wq": _pack(W_qkv[:, cs].astype(f16)),
            "wk": _pack(W_qkv[:, D + CPC * g : D + CPC * (g + 1)].astype(f16)),
            "wv": _pack(W_qkv[:, 2 * D + CPC * g : 2 * D + CPC * (g + 1)].astype(f16)),
            "wo": _pack(W_lin[cs, :].astype(f16)),
            "bq": b_qkv[cs].astype(np.float32),
            "bk": b_qkv[D + CPC * g : D + CPC * (g + 1)].astype(np.float32),
            "bv": b_qkv[2 * D + CPC * g : 2 * D + CPC * (g + 1)].astype(np.float32),
        })
    return in_maps


def kernel(x, W_qkv, b_qkv, W_lin, b_lin, _trace=False):
    nc = _compiled()
    in_maps = make_in_maps(x, W_qkv, b_qkv, W_lin)
    res = run_bass_kernel_spmd(nc, in_maps, core_ids=list(range(N_CORES)),
                               trace=_trace)
    parts = [r["out"] for r in res.results]
    out = np.empty((B, L, D), dtype=np.float32)
    for b in range(B):
        out[b] = (parts[2 * b].astype(np.float32)
                  + parts[2 * b + 1].astype(np.float32)
                  + b_lin.astype(np.float32))
    if _trace:
        return out, res
    return out


# revision 33
# speedup vs baseline: 1.0441x; 1.0441x over previous
"""Multi-head self-attention (B=4, L=2048, D=1024, H=16) on 8 trn2 cores.

Sharding: DP=4 over batch x TP=2 over heads (8 heads/core). Each core:
  QT/KT = W.T @ x.T projections (c on partitions), V natural layout,
  S^T = K Q^T per head with keys on partitions, exp on ACT, causal via
  tile skipping + gpsimd affine_select zeroing of P on diagonal tiles,
  ctx^T = V'.T @ P^T with a ones-column in V' producing the softmax
  denominator row, normalization via fast-approx reciprocal (DVE) +
  gpsimd partition_broadcast + DVE multiply, then the output projection
  produces a per-core partial [L, D] in f16. Host sums the TP pair
  partials + b_lin in f32.

Phases are interleaved per 512-query block (proj -> attn -> prev
block's outproj) to keep the PE busy continuously (HAM stays warm).
"""

import sys

if "/opt/trn_rl_repo" not in sys.path:
    sys.path.insert(0, "/opt/trn_rl_repo")

import numpy as np

import concourse.bass as bass
import concourse.tile as tile
from concourse import bacc, mybir
from concourse.bass_utils import run_bass_kernel_spmd

B, L, D = 4, 2048, 1024
H, HD = 16, 64
HPC = H // 2          # heads per core (TP=2)
CPC = HPC * HD        # head-dim columns per core = 512
N_CORES = 8

F32 = mybir.dt.float32
F16 = mybir.dt.float16

NB = 4                # 512-wide i/l blocks
BW = L // NB          # 512
NT = L // 128         # 16 j/l tiles of 128
DT = D // 128         # 8 d-tiles
CT = CPC // 128       # 4 c-tiles per core


def build(tc, io):
    nc = tc.nc
    # host pre-packs weights/x into SBUF layout: DMA lines are 4-32KB
    xbT = io["xbT"]       # [128, 8, 2048] f16 (packed d-tiles)
    wq = io["wq"]         # [128, 8, 512] f16
    wk = io["wk"]
    wv = io["wv"]
    wo = io["wo"]         # [128, 4, 1024] f16
    bq = io["bq"].rearrange("(t p) -> p t", p=128)            # [128,4] f32
    bk = io["bk"].rearrange("(t p) -> p t", p=128)
    bv = io["bv"].rearrange("(t p) -> p t", p=128)
    out = io["out"].rearrange("(t p) e -> t p e", p=128)      # [16,128,1024] f16

    singles = tc.alloc_tile_pool(name="singles", bufs=1)
    work = tc.alloc_tile_pool(name="work", bufs=4)
    psum = tc.alloc_tile_pool(name="psum", bufs=2, space="PSUM")

    # --- resident tensors (packed DMAs: 4-8KB contiguous per partition) -----
    wq_sb = singles.tile([128, DT, CPC], F16)
    wk_sb = singles.tile([128, DT, CPC], F16)
    wv_sb = singles.tile([128, DT, CPC], F16)
    wo_sb = singles.tile([128, CT, D], F16)
    # x packed block-major: one full-rate 1MB DMA delivers a whole
    # 512-query block, so proj(0) streams without x stalls
    x_sb = singles.tile([128, NB, DT, BW], F16)
    nc.sync.dma_start(out=wq_sb, in_=wq)
    nc.sync.dma_start(out=x_sb[:, 0], in_=xbT[:, 0])
    nc.sync.dma_start(out=wk_sb, in_=wk)
    nc.sync.dma_start(out=wv_sb, in_=wv)
    for blk in range(1, NB):
        nc.sync.dma_start(out=x_sb[:, blk], in_=xbT[:, blk])
    nc.sync.dma_start(out=wo_sb, in_=wo)
    bq_sb = singles.tile([128, CT], F32)
    bk_sb = singles.tile([128, CT], F32)
    bv_sb = singles.tile([128, CT], F32)
    nc.sync.dma_start(out=bq_sb, in_=bq)
    nc.sync.dma_start(out=bk_sb, in_=bk)
    nc.sync.dma_start(out=bv_sb, in_=bv)

    qt_sb = singles.tile([128, CT, L], F16)    # Q^T  [c, i]
    kt_sb = singles.tile([128, CT, L], F16)    # K^T  [c, j]
    vp_sb = singles.tile([128, NT, HPC, HD + 1], F16)   # V' [j, (h, d'|1)]
    cxt_sb = singles.tile([128, CT, L], F16)   # normalized ctx^T [d', i]
    nc.gpsimd.memset(vp_sb[:, :, :, HD : HD + 1], 1.0)
    # 1 where free-idx >= partition-idx (kept region of a diagonal tile)
    mask2_sb = singles.tile([128, 2, BW], F16)
    nc.gpsimd.memset(mask2_sb, 1.0)
    nc.gpsimd.affine_select(
        out=mask2_sb, in_=mask2_sb, compare_op=mybir.AluOpType.is_ge,
        fill=0.0, base=0, pattern=[[0, 2], [1, BW]], channel_multiplier=-1)

    def proj(blk):
        xs = x_sb[:, blk]   # [128, DT, BW]
        for w_sb, b_sb, t_sb, nm in ((wq_sb, bq_sb, qt_sb, "q"),
                                     (wk_sb, bk_sb, kt_sb, "k")):
            for cp in range(2):
                ps_p = [psum.tile([128, BW], F32, tag="pp", bufs=2,
                                  name=f"ps{nm}{blk}{cp}_{ci}") for ci in range(2)]
                for dt in range(DT):
                    for ci in range(2):
                        nc.tensor.matmul(
                            ps_p[ci],
                            lhsT=w_sb[:, dt, (2 * cp + ci) * 128 : (2 * cp + ci + 1) * 128],
                            rhs=xs[:, dt], start=(dt == 0), stop=(dt == DT - 1))
                for ci in range(2):
                    ct = 2 * cp + ci
                    nc.vector.tensor_scalar_add(
                        out=t_sb[:, ct, blk * BW : (blk + 1) * BW],
                        in0=ps_p[ci], scalar1=b_sb[:, ct : ct + 1])
        for j in range(4):
            lt = 4 * blk + j
            ps_v = psum.tile([128, CPC], F32, tag="pp", bufs=2,
                             name=f"psv{lt}")
            for dt in range(DT):
                nc.tensor.matmul(
                    ps_v, lhsT=xs[:, dt, j * 128 : (j + 1) * 128],
                    rhs=wv_sb[:, dt, :], start=(dt == 0), stop=(dt == DT - 1))
            # V bias is folded in after normalization (bv * sum(P) = bv).
            nc.vector.tensor_copy(
                out=vp_sb[:, lt, :, 0:HD],
                in_=ps_v.rearrange("p (h d) -> p h d", h=HPC))

    def attn(hp, ib):
        ctx_ps = [psum.tile([128, BW], F32, tag="ctx", bufs=2,
                            name=f"cx{hp}_{ib}_{hh}") for hh in range(2)]
        ntj = 4 * ib + 4
        for tj in range(ntj):
            k = tj - 4 * ib
            off = 128 * k if k >= 0 else 0
            w = BW - off
            st = psum.tile([128, 2, BW], F32, tag="big", bufs=2,
                           name=f"st{hp}_{ib}_{tj}")
            pt = work.tile([128, 2, BW], F16, tag="pt", name=f"pt{hp}_{ib}_{tj}")
            for hh in range(2):
                nc.tensor.matmul(
                    st[:, hh, off:], skip_group_check=True, start=True, stop=True,
                    lhsT=kt_sb[64 * hh : 64 * hh + 64, hp,
                               tj * 128 : (tj + 1) * 128],
                    rhs=qt_sb[64 * hh : 64 * hh + 64, hp,
                              ib * BW + off : (ib + 1) * BW])
            nc.scalar.activation(
                out=pt[:, :, off:], in_=st[:, :, off:],
                func=mybir.ActivationFunctionType.Exp, scale=0.125)
            if k >= 0:
                # zero the j>i triangle of this diagonal tile (query-local
                # index f vs key-local index p share the same off origin)
                nc.vector.tensor_tensor(
                    out=pt[:, :, off:], in0=pt[:, :, off:],
                    in1=mask2_sb[:, :, 0:w], op=mybir.AluOpType.mult)
            for hh in range(2):
                nc.tensor.matmul(
                    ctx_ps[hh][0 : HD + 1, off:], skip_group_check=True,
                    lhsT=vp_sb[:, tj, 2 * hp + hh, :],
                    rhs=pt[:, hh, off:], start=(tj == 0), stop=(tj == ntj - 1))
        for hh in range(2):
            # contiguous partition-0 tiles only: the custom DVE/gpsimd ops
            # drop input partition offsets on HW, so stage the denominator
            # row at partition 0 with a standard copy first.
            dcp = work.tile([1, BW], F32, tag="dcp", name=f"dc{hp}{ib}{hh}")
            rden = work.tile([1, BW], F32, tag="rden", name=f"rd{hp}{ib}{hh}")
            bcst = work.tile([64, BW], F32, tag="bcs", name=f"bs{hp}{ib}{hh}")
            nc.vector.tensor_copy(out=dcp, in_=ctx_ps[hh][HD : HD + 1, :])
            nc.vector.reciprocal_approx_fast(out=rden, in_=dcp)
            nc.gpsimd.partition_broadcast(out_ap=bcst, in_ap=rden)
            dst = cxt_sb[64 * hh : 64 * hh + 64, hp, ib * BW : (ib + 1) * BW]
            nc.vector.tensor_tensor(
                out=dst, in0=ctx_ps[hh][0:HD, :], in1=bcst,
                op=mybir.AluOpType.mult)
            nc.vector.tensor_scalar_add(
                out=dst, in0=dst,
                scalar1=bv_sb[64 * hh : 64 * hh + 64, hp : hp + 1])

    def outproj(it):
        ps_o = [psum.tile([128, 512], F32, tag="pp", bufs=2,
                          name=f"po{it}_{eb}") for eb in range(2)]
        for ct in range(CT):
            for eb in range(2):
                nc.tensor.matmul(
                    ps_o[eb],
                    lhsT=cxt_sb[:, ct, it * 128 : (it + 1) * 128],
                    rhs=wo_sb[:, ct, eb * 512 : (eb + 1) * 512],
                    start=(ct == 0), stop=(ct == CT - 1))
        o_sb = work.tile([128, D], F16, tag="osb", name=f"os{it}")
        for eb in range(2):
            nc.vector.tensor_copy(out=o_sb[:, eb * 512 : (eb + 1) * 512],
                                  in_=ps_o[eb])
        nc.sync.dma_start(out=out[it], in_=o_sb)

    # software pipeline: outproj trails attn by one block so its PE work
    # and PSUM traffic never gate the next block's projections.
    for blk in range(NB):
        proj(blk)
        for hp in range(CT):
            attn(hp, blk)
        if blk > 0:
            for j in range(4):
                outproj(4 * (blk - 1) + j)
    for j in range(4):
        outproj(4 * (NB - 1) + j)

    for p in (psum, work, singles):
        p.release()


_CACHE = {}


def _compiled():
    if "nc" in _CACHE:
        return _CACHE["nc"]
    nc = bacc.Bacc("TRN2", target_bir_lowering=False, debug=False)
    io = {
        "xbT": nc.dram_tensor("xbT", [128, NB, DT, BW], F16,
                              kind="ExternalInput").ap(),
        "wq": nc.dram_tensor("wq", [128, DT, CPC], F16, kind="ExternalInput").ap(),
        "wk": nc.dram_tensor("wk", [128, DT, CPC], F16, kind="ExternalInput").ap(),
        "wv": nc.dram_tensor("wv", [128, DT, CPC], F16, kind="ExternalInput").ap(),
        "wo": nc.dram_tensor("wo", [128, CT, D], F16, kind="ExternalInput").ap(),
        "bq": nc.dram_tensor("bq", [CPC], F32, kind="ExternalInput").ap(),
        "bk": nc.dram_tensor("bk", [CPC], F32, kind="ExternalInput").ap(),
        "bv": nc.dram_tensor("bv", [CPC], F32, kind="ExternalInput").ap(),
        "out": nc.dram_tensor("out", [L, D], F16, kind="ExternalOutput").ap(),
    }
    with tile.TileContext(nc) as tc:
        build(tc, io)
    nc.compile()
    _CACHE["nc"] = nc
    return nc


def _pack(a, p=128):
    """[T*p, N] -> [p, T, N] (SBUF layout: partition-major)."""
    t = a.shape[0] // p
    return np.ascontiguousarray(
        a.reshape(t, p, a.shape[1]).transpose(1, 0, 2))


def _pack_x(xT):
    """[DT*128, NB*BW] -> [128, NB, DT, BW] (block-major SBUF layout)."""
    return np.ascontiguousarray(
        xT.reshape(DT, 128, NB, BW).transpose(1, 2, 0, 3))


def make_in_maps(x, W_qkv, b_qkv, W_lin):
    f16 = mybir.dt.np(F16)
    in_maps = []
    for c in range(N_CORES):
        b, g = divmod(c, 2)
        cs = slice(CPC * g, CPC * (g + 1))
        in_maps.append({
            "xbT": _pack_x(np.ascontiguousarray(x[b].T).astype(f16)),
            "wq": _pack(W_qkv[:, cs].astype(f16)),
            "wk": _pack(W_qkv[:, D + CPC * g : D + CPC * (g + 1)].astype(f16)),
            "wv": _pack(W_qkv[:, 2 * D + CPC * g : 2 * D + CPC * (g + 1)].astype(f16)),
            "wo": _pack(W_lin[cs, :].astype(f16)),
            "bq": b_qkv[cs].astype(np.float32),
            "bk": b_qkv[D + CPC * g : D + CPC * (g + 1)].astype(np.float32),
            "bv": b_qkv[2 * D + CPC * g : 2 * D + CPC * (g + 1)].astype(np.float32),
        })
    return in_maps


def kernel(x, W_qkv, b_qkv, W_lin, b_lin, _trace=False):
    nc = _compiled()
    in_maps = make_in_maps(x, W_qkv, b_qkv, W_lin)
    res = run_bass_kernel_spmd(nc, in_maps, core_ids=list(range(N_CORES)),
                               trace=_trace)
    parts = [r["out"] for r in res.results]
    out = np.empty((B, L, D), dtype=np.float32)
    for b in range(B):
        out[b] = (parts[2 * b].astype(np.float32)
                  + parts[2 * b + 1].astype(np.float32)
                  + b_lin.astype(np.float32))
    if _trace:
        return out, res
    return out


# revision 34
# speedup vs baseline: 1.0443x; 1.0002x over previous
"""Multi-head self-attention (B=4, L=2048, D=1024, H=16) on 8 trn2 cores.

Sharding: DP=4 over batch x TP=2 over heads (8 heads/core). Each core:
  QT/KT = W.T @ x.T projections (c on partitions), V natural layout,
  S^T = K Q^T per head with keys on partitions, exp on ACT, causal via
  tile skipping + DVE mask multiply on diagonal tiles, ctx^T = V'.T @
  P^T with a ones-column in V' producing the softmax denominator row,
  normalization via fast-approx reciprocal (DVE, ~5x the iterative
  divide) + gpsimd partition_broadcast + DVE multiply, then the output
  projection produces a per-core partial [L, D] in f16. Host sums the
  TP pair partials + b_lin in f32.

Phases are interleaved per 512-query block (proj -> attn -> previous
block's outproj) to keep the PE continuously busy (HAM stays warm).
Host pre-packs x/weights into partition-major SBUF layout so every
input DMA moves 4-32KB contiguous per partition (full HBM rate); x is
additionally block-major so proj(0) starts after ~2 DMAs.

HW-vs-CoreSim gotchas baked into this design (sim passes either way,
HW does not): custom DVE ops (reciprocal_approx_fast) drop the input
partition offset -> stage the denominator row to a partition-0 tile
first; gpsimd ucode (partition_broadcast/affine_select) crashes on
strided APs -> contiguous whole-tile APs only; unaligned partition
bases (e.g. psum[1:65]) crash the walrus backend.
"""

import sys

if "/opt/trn_rl_repo" not in sys.path:
    sys.path.insert(0, "/opt/trn_rl_repo")

import numpy as np

import concourse.bass as bass
import concourse.tile as tile
from concourse import bacc, mybir
from concourse.bass_utils import run_bass_kernel_spmd

B, L, D = 4, 2048, 1024
H, HD = 16, 64
HPC = H // 2          # heads per core (TP=2)
CPC = HPC * HD        # head-dim columns per core = 512
N_CORES = 8

F32 = mybir.dt.float32
F16 = mybir.dt.float16

NB = 4                # 512-wide i/l blocks
BW = L // NB          # 512
NT = L // 128         # 16 j/l tiles of 128
DT = D // 128         # 8 d-tiles
CT = CPC // 128       # 4 c-tiles per core


def build(tc, io):
    nc = tc.nc
    # host pre-packs weights/x into SBUF layout: DMA lines are 4-32KB
    xbT = io["xbT"]       # [128, 8, 2048] f16 (packed d-tiles)
    wq = io["wq"]         # [128, 8, 512] f16
    wk = io["wk"]
    wv = io["wv"]
    wo = io["wo"]         # [128, 4, 1024] f16
    bq = io["bq"].rearrange("(t p) -> p t", p=128)            # [128,4] f32
    bk = io["bk"].rearrange("(t p) -> p t", p=128)
    bv = io["bv"].rearrange("(t p) -> p t", p=128)
    out = io["out"].rearrange("(t p) e -> t p e", p=128)      # [16,128,1024] f16

    singles = tc.alloc_tile_pool(name="singles", bufs=1)
    work = tc.alloc_tile_pool(name="work", bufs=4)
    psum = tc.alloc_tile_pool(name="psum", bufs=2, space="PSUM")

    # --- resident tensors (packed DMAs: 4-8KB contiguous per partition) -----
    wq_sb = singles.tile([128, DT, CPC], F16)
    wk_sb = singles.tile([128, DT, CPC], F16)
    wv_sb = singles.tile([128, DT, CPC], F16)
    wo_sb = singles.tile([128, CT, D], F16)
    # x packed block-major: one full-rate 1MB DMA delivers a whole
    # 512-query block, so proj(0) streams without x stalls
    x_sb = singles.tile([128, NB, DT, BW], F16)
    nc.sync.dma_start(out=wq_sb, in_=wq)
    nc.sync.dma_start(out=x_sb[:, 0], in_=xbT[:, 0])
    nc.sync.dma_start(out=wk_sb, in_=wk)
    nc.sync.dma_start(out=wv_sb, in_=wv)
    for blk in range(1, NB):
        nc.sync.dma_start(out=x_sb[:, blk], in_=xbT[:, blk])
    nc.sync.dma_start(out=wo_sb, in_=wo)
    bq_sb = singles.tile([128, CT], F32)
    bk_sb = singles.tile([128, CT], F32)
    bv_sb = singles.tile([128, CT], F32)
    nc.sync.dma_start(out=bq_sb, in_=bq)
    nc.sync.dma_start(out=bk_sb, in_=bk)
    nc.sync.dma_start(out=bv_sb, in_=bv)

    qt_sb = singles.tile([128, CT, L], F16)    # Q^T  [c, i]
    kt_sb = singles.tile([128, CT, L], F16)    # K^T  [c, j]
    vp_sb = singles.tile([128, NT, HPC, HD + 1], F16)   # V' [j, (h, d'|1)]
    cxt_sb = singles.tile([128, CT, L], F16)   # normalized ctx^T [d', i]
    nc.gpsimd.memset(vp_sb[:, :, :, HD : HD + 1], 1.0)
    # 1 where free-idx >= partition-idx (kept region of a diagonal tile)
    mask2_sb = singles.tile([128, 2, BW], F16)
    nc.gpsimd.memset(mask2_sb, 1.0)
    nc.gpsimd.affine_select(
        out=mask2_sb, in_=mask2_sb, compare_op=mybir.AluOpType.is_ge,
        fill=0.0, base=0, pattern=[[0, 2], [1, BW]], channel_multiplier=-1)

    def proj(blk):
        xs = x_sb[:, blk]   # [128, DT, BW]
        for w_sb, b_sb, t_sb, nm in ((wq_sb, bq_sb, qt_sb, "q"),
                                     (wk_sb, bk_sb, kt_sb, "k")):
            for cp in range(2):
                ps_p = [psum.tile([128, BW], F32, tag="pp", bufs=2,
                                  name=f"ps{nm}{blk}{cp}_{ci}") for ci in range(2)]
                for dt in range(DT):
                    for ci in range(2):
                        nc.tensor.matmul(
                            ps_p[ci],
                            lhsT=w_sb[:, dt, (2 * cp + ci) * 128 : (2 * cp + ci + 1) * 128],
                            rhs=xs[:, dt], start=(dt == 0), stop=(dt == DT - 1))
                for ci in range(2):
                    ct = 2 * cp + ci
                    nc.vector.tensor_scalar_add(
                        out=t_sb[:, ct, blk * BW : (blk + 1) * BW],
                        in0=ps_p[ci], scalar1=b_sb[:, ct : ct + 1])
        for j in range(4):
            lt = 4 * blk + j
            ps_v = psum.tile([128, CPC], F32, tag="pp", bufs=2,
                             name=f"psv{lt}")
            for dt in range(DT):
                nc.tensor.matmul(
                    ps_v, lhsT=xs[:, dt, j * 128 : (j + 1) * 128],
                    rhs=wv_sb[:, dt, :], start=(dt == 0), stop=(dt == DT - 1))
            # V bias is folded in after normalization (bv * sum(P) = bv).
            nc.vector.tensor_copy(
                out=vp_sb[:, lt, :, 0:HD],
                in_=ps_v.rearrange("p (h d) -> p h d", h=HPC))

    def attn(hp, ib):
        ctx_ps = [psum.tile([128, BW], F32, tag="ctx", bufs=2,
                            name=f"cx{hp}_{ib}_{hh}") for hh in range(2)]
        ntj = 4 * ib + 4
        for tj in range(ntj):
            k = tj - 4 * ib
            off = 128 * k if k >= 0 else 0
            w = BW - off
            st = psum.tile([128, 2, BW], F32, tag="big", bufs=2,
                           name=f"st{hp}_{ib}_{tj}")
            pt = work.tile([128, 2, BW], F16, tag="pt", name=f"pt{hp}_{ib}_{tj}")
            for hh in range(2):
                nc.tensor.matmul(
                    st[:, hh, off:], skip_group_check=True, start=True, stop=True,
                    lhsT=kt_sb[64 * hh : 64 * hh + 64, hp,
                               tj * 128 : (tj + 1) * 128],
                    rhs=qt_sb[64 * hh : 64 * hh + 64, hp,
                              ib * BW + off : (ib + 1) * BW])
            nc.scalar.activation(
                out=pt[:, :, off:], in_=st[:, :, off:],
                func=mybir.ActivationFunctionType.Exp, scale=0.125)
            if k >= 0:
                # zero the j>i triangle of this diagonal tile (query-local
                # index f vs key-local index p share the same off origin)
                nc.vector.tensor_tensor(
                    out=pt[:, :, off:], in0=pt[:, :, off:],
                    in1=mask2_sb[:, :, 0:w], op=mybir.AluOpType.mult)
            for hh in range(2):
                nc.tensor.matmul(
                    ctx_ps[hh][0 : HD + 1, off:], skip_group_check=True,
                    lhsT=vp_sb[:, tj, 2 * hp + hh, :],
                    rhs=pt[:, hh, off:], start=(tj == 0), stop=(tj == ntj - 1))
        for hh in range(2):
            # contiguous partition-0 tiles only: the custom DVE/gpsimd ops
            # drop input partition offsets on HW, so stage the denominator
            # row at partition 0 with a standard copy first.
            dcp = work.tile([1, BW], F32, tag="dcp", name=f"dc{hp}{ib}{hh}")
            rden = work.tile([1, BW], F32, tag="rden", name=f"rd{hp}{ib}{hh}")
            bcst = work.tile([64, BW], F32, tag="bcs", name=f"bs{hp}{ib}{hh}")
            nc.vector.tensor_copy(out=dcp, in_=ctx_ps[hh][HD : HD + 1, :])
            nc.vector.reciprocal_approx_fast(out=rden, in_=dcp)
            nc.gpsimd.partition_broadcast(out_ap=bcst, in_ap=rden)
            dst = cxt_sb[64 * hh : 64 * hh + 64, hp, ib * BW : (ib + 1) * BW]
            nc.vector.tensor_tensor(
                out=dst, in0=ctx_ps[hh][0:HD, :], in1=bcst,
                op=mybir.AluOpType.mult)
            nc.vector.tensor_scalar_add(
                out=dst, in0=dst,
                scalar1=bv_sb[64 * hh : 64 * hh + 64, hp : hp + 1])

    def outproj(it):
        ps_o = [psum.tile([128, 512], F32, tag="pp", bufs=2,
                          name=f"po{it}_{eb}") for eb in range(2)]
        for ct in range(CT):
            for eb in range(2):
                nc.tensor.matmul(
                    ps_o[eb],
                    lhsT=cxt_sb[:, ct, it * 128 : (it + 1) * 128],
                    rhs=wo_sb[:, ct, eb * 512 : (eb + 1) * 512],
                    start=(ct == 0), stop=(ct == CT - 1))
        o_sb = work.tile([128, D], F16, tag="osb", name=f"os{it}")
        for eb in range(2):
            nc.vector.tensor_copy(out=o_sb[:, eb * 512 : (eb + 1) * 512],
                                  in_=ps_o[eb])
        nc.sync.dma_start(out=out[it], in_=o_sb)

    # software pipeline: outproj trails attn by one block so its PE work
    # and PSUM traffic never gate the next block's projections.
    for blk in range(NB):
        proj(blk)
        for hp in range(CT):
            attn(hp, blk)
        if blk > 0:
            for j in range(4):
                outproj(4 * (blk - 1) + j)
    for j in range(4):
        outproj(4 * (NB - 1) + j)

    for p in (psum, work, singles):
        p.release()


_CACHE = {}


def _compiled():
    if "nc" in _CACHE:
        return _CACHE["nc"]
    nc = bacc.Bacc("TRN2", target_bir_lowering=False, debug=False)
    io = {
        "xbT": nc.dram_tensor("xbT", [128, NB, DT, BW], F16,
                              kind="ExternalInput").ap(),
        "wq": nc.dram_tensor("wq", [128, DT, CPC], F16, kind="ExternalInput").ap(),
        "wk": nc.dram_tensor("wk", [128, DT, CPC], F16, kind="ExternalInput").ap(),
        "wv": nc.dram_tensor("wv", [128, DT, CPC], F16, kind="ExternalInput").ap(),
        "wo": nc.dram_tensor("wo", [128, CT, D], F16, kind="ExternalInput").ap(),
        "bq": nc.dram_tensor("bq", [CPC], F32, kind="ExternalInput").ap(),
        "bk": nc.dram_tensor("bk", [CPC], F32, kind="ExternalInput").ap(),
        "bv": nc.dram_tensor("bv", [CPC], F32, kind="ExternalInput").ap(),
        "out": nc.dram_tensor("out", [L, D], F16, kind="ExternalOutput").ap(),
    }
    with tile.TileContext(nc) as tc:
        build(tc, io)
    nc.compile()
    _CACHE["nc"] = nc
    return nc


def _pack(a, p=128):
    """[T*p, N] -> [p, T, N] (SBUF layout: partition-major)."""
    t = a.shape[0] // p
    return np.ascontiguousarray(
        a.reshape(t, p, a.shape[1]).transpose(1, 0, 2))


def _pack_x(xT):
    """[DT*128, NB*BW] -> [128, NB, DT, BW] (block-major SBUF layout)."""
    return np.ascontiguousarray(
        xT.reshape(DT, 128, NB, BW).transpose(1, 2, 0, 3))


def make_in_maps(x, W_qkv, b_qkv, W_lin):
    f16 = mybir.dt.np(F16)
    in_maps = []
    for c in range(N_CORES):
        b, g = divmod(c, 2)
        cs = slice(CPC * g, CPC * (g + 1))
        in_maps.append({
            "xbT": _pack_x(np.ascontiguousarray(x[b].T).astype(f16)),
            "wq": _pack(W_qkv[:, cs].astype(f16)),
            "wk": _pack(W_qkv[:, D + CPC * g : D + CPC * (g + 1)].astype(f16)),
            "wv": _pack(W_qkv[:, 2 * D + CPC * g : 2 * D + CPC * (g + 1)].astype(f16)),
            "wo": _pack(W_lin[cs, :].astype(f16)),
            "bq": b_qkv[cs].astype(np.float32),
            "bk": b_qkv[D + CPC * g : D + CPC * (g + 1)].astype(np.float32),
            "bv": b_qkv[2 * D + CPC * g : 2 * D + CPC * (g + 1)].astype(np.float32),
        })
    return in_maps


def kernel(x, W_qkv, b_qkv, W_lin, b_lin, _trace=False):
    nc = _compiled()
    in_maps = make_in_maps(x, W_qkv, b_qkv, W_lin)
    res = run_bass_kernel_spmd(nc, in_maps, core_ids=list(range(N_CORES)),
                               trace=_trace)
    parts = [r["out"] for r in res.results]
    out = np.empty((B, L, D), dtype=np.float32)
    for b in range(B):
        out[b] = (parts[2 * b].astype(np.float32)
                  + parts[2 * b + 1].astype(np.float32)
                  + b_lin.astype(np.float32))
    if _trace:
        return out, res
    return out


# revision 39
# speedup vs baseline: 1.0916x; 1.0453x over previous
"""Multi-head self-attention (B=4, L=2048, D=1024, H=16) on 8 trn2 cores.

Sharding: DP=4 over batch x TP=2 over heads (8 heads/core). Each core:
  QT/KT = W.T @ x.T projections (c on partitions), V natural layout,
  S^T = K Q^T per head with keys on partitions, exp on ACT, causal via
  tile skipping + DVE mask multiply on diagonal tiles, ctx^T = V'.T @
  P^T with a ones-column in V' producing the softmax denominator row,
  normalization via fast-approx reciprocal (DVE, ~5x the iterative
  divide) + gpsimd partition_broadcast + DVE multiply, then the output
  projection produces a per-core partial [L, D] in f16. Host sums the
  TP pair partials + b_lin in f32.

Phases are interleaved per 512-query block (proj -> attn -> previous
block's outproj) to keep the PE continuously busy (HAM stays warm).
Host pre-packs x/weights into partition-major SBUF layout so every
input DMA moves 4-32KB contiguous per partition (full HBM rate); x is
additionally block-major so proj(0) starts after ~2 DMAs.

HW-vs-CoreSim gotchas baked into this design (sim passes either way,
HW does not): custom DVE ops (reciprocal_approx_fast) drop the input
partition offset -> stage the denominator row to a partition-0 tile
first; gpsimd ucode (partition_broadcast/affine_select) crashes on
strided APs -> contiguous whole-tile APs only; unaligned partition
bases (e.g. psum[1:65]) crash the walrus backend.
"""

import sys

if "/opt/trn_rl_repo" not in sys.path:
    sys.path.insert(0, "/opt/trn_rl_repo")

import numpy as np

import concourse.bass as bass
import concourse.tile as tile
from concourse import bacc, mybir
from concourse.bass_utils import run_bass_kernel_spmd

B, L, D = 4, 2048, 1024
H, HD = 16, 64
HPC = H // 2          # heads per core (TP=2)
CPC = HPC * HD        # head-dim columns per core = 512
N_CORES = 8

F32 = mybir.dt.float32
F16 = mybir.dt.float16

NB = 4                # 512-wide i/l blocks
BW = L // NB          # 512
NT = L // 128         # 16 j/l tiles of 128
DT = D // 128         # 8 d-tiles
CT = CPC // 128       # 4 c-tiles per core


def build(tc, io):
    nc = tc.nc
    # host pre-packs weights/x into SBUF layout: DMA lines are 4-32KB
    xbT = io["xbT"]       # [128, 8, 2048] f16 (packed d-tiles)
    wq = io["wq"]         # [128, 8, 512] f16
    wk = io["wk"]
    wv = io["wv"]
    wo = io["wo"]         # [128, 4, 1024] f16
    bqkv = io["bqkv"]     # [128, 12] f32 (host-packed bq|bk|bv, ct-major)
    out = io["out"].rearrange("(t p) e -> t p e", p=128)      # [16,128,1024] f16

    singles = tc.alloc_tile_pool(name="singles", bufs=1)
    work = tc.alloc_tile_pool(name="work", bufs=4)
    psum = tc.alloc_tile_pool(name="psum", bufs=2, space="PSUM")

    # --- resident tensors (packed DMAs: 4-8KB contiguous per partition) -----
    wq_sb = singles.tile([128, DT, CPC], F16)
    wk_sb = singles.tile([128, DT, CPC], F16)
    wv_sb = singles.tile([128, DT, CPC], F16)
    wo_sb = singles.tile([128, CT, D], F16)
    # x packed block-major: one full-rate 1MB DMA delivers a whole
    # 512-query block, so proj(0) streams without x stalls
    x_sb = singles.tile([128, NB, DT, BW], F16)
    # biases first: tiny (6KB) but a naive unpacked layout costs 3x9us of
    # 4-byte-descriptor DMA that stalled the whole first block
    bqkv_sb = singles.tile([128, 3 * CT], F32)
    nc.sync.dma_start(out=bqkv_sb, in_=bqkv)
    bq_sb = bqkv_sb[:, 0:CT]
    bk_sb = bqkv_sb[:, CT : 2 * CT]
    bv_sb = bqkv_sb[:, 2 * CT : 3 * CT]
    nc.sync.dma_start(out=wq_sb, in_=wq)
    nc.sync.dma_start(out=x_sb[:, 0], in_=xbT[:, 0])
    nc.sync.dma_start(out=wk_sb, in_=wk)
    nc.sync.dma_start(out=wv_sb, in_=wv)
    for blk in range(1, NB):
        nc.sync.dma_start(out=x_sb[:, blk], in_=xbT[:, blk])
    nc.sync.dma_start(out=wo_sb, in_=wo)

    qt_sb = singles.tile([128, CT, L], F16)    # Q^T  [c, i]
    kt_sb = singles.tile([128, CT, L], F16)    # K^T  [c, j]
    vp_sb = singles.tile([128, NT, HPC, HD + 1], F16)   # V' [j, (h, d'|1)]
    cxt_sb = singles.tile([128, CT, L], F16)   # normalized ctx^T [d', i]
    nc.gpsimd.memset(vp_sb[:, :, :, HD : HD + 1], 1.0)
    # 1 where free-idx >= partition-idx (kept region of a diagonal tile)
    mask2_sb = singles.tile([128, 2, BW], F16)
    nc.gpsimd.memset(mask2_sb, 1.0)
    nc.gpsimd.affine_select(
        out=mask2_sb, in_=mask2_sb, compare_op=mybir.AluOpType.is_ge,
        fill=0.0, base=0, pattern=[[0, 2], [1, BW]], channel_multiplier=-1)

    def proj(blk):
        xs = x_sb[:, blk]   # [128, DT, BW]
        for w_sb, b_sb, t_sb, nm in ((wq_sb, bq_sb, qt_sb, "q"),
                                     (wk_sb, bk_sb, kt_sb, "k")):
            for cp in range(2):
                ps_p = [psum.tile([128, BW], F32, tag="pp", bufs=2,
                                  name=f"ps{nm}{blk}{cp}_{ci}") for ci in range(2)]
                for dt in range(DT):
                    for ci in range(2):
                        nc.tensor.matmul(
                            ps_p[ci],
                            lhsT=w_sb[:, dt, (2 * cp + ci) * 128 : (2 * cp + ci + 1) * 128],
                            rhs=xs[:, dt], start=(dt == 0), stop=(dt == DT - 1))
                for ci in range(2):
                    ct = 2 * cp + ci
                    nc.vector.tensor_scalar_add(
                        out=t_sb[:, ct, blk * BW : (blk + 1) * BW],
                        in0=ps_p[ci], scalar1=b_sb[:, ct : ct + 1])
        for j in range(4):
            lt = 4 * blk + j
            ps_v = psum.tile([128, CPC], F32, tag="pp", bufs=2,
                             name=f"psv{lt}")
            for dt in range(DT):
                nc.tensor.matmul(
                    ps_v, lhsT=xs[:, dt, j * 128 : (j + 1) * 128],
                    rhs=wv_sb[:, dt, :], start=(dt == 0), stop=(dt == DT - 1))
            # V bias is folded in after normalization (bv * sum(P) = bv).
            nc.vector.tensor_copy(
                out=vp_sb[:, lt, :, 0:HD],
                in_=ps_v.rearrange("p (h d) -> p h d", h=HPC))

    def attn(hp, ib):
        ctx_ps = [psum.tile([128, BW], F32, tag="ctx", bufs=2,
                            name=f"cx{hp}_{ib}_{hh}") for hh in range(2)]
        ntj = 4 * ib + 4
        for tj in range(ntj):
            k = tj - 4 * ib
            off = 128 * k if k >= 0 else 0
            w = BW - off
            st = psum.tile([128, 2, BW], F32, tag="big", bufs=2,
                           name=f"st{hp}_{ib}_{tj}")
            pt = work.tile([128, 2, BW], F16, tag="pt", name=f"pt{hp}_{ib}_{tj}")
            for hh in range(2):
                nc.tensor.matmul(
                    st[:, hh, off:], skip_group_check=True, start=True, stop=True,
                    lhsT=kt_sb[64 * hh : 64 * hh + 64, hp,
                               tj * 128 : (tj + 1) * 128],
                    rhs=qt_sb[64 * hh : 64 * hh + 64, hp,
                              ib * BW + off : (ib + 1) * BW])
            nc.scalar.activation(
                out=pt[:, :, off:], in_=st[:, :, off:],
                func=mybir.ActivationFunctionType.Exp, scale=0.125)
            if k >= 0:
                # zero the j>i triangle of this diagonal tile (query-local
                # index f vs key-local index p share the same off origin)
                nc.vector.tensor_tensor(
                    out=pt[:, :, off:], in0=pt[:, :, off:],
                    in1=mask2_sb[:, :, 0:w], op=mybir.AluOpType.mult)
            for hh in range(2):
                nc.tensor.matmul(
                    ctx_ps[hh][0 : HD + 1, off:], skip_group_check=True,
                    lhsT=vp_sb[:, tj, 2 * hp + hh, :],
                    rhs=pt[:, hh, off:], start=(tj == 0), stop=(tj == ntj - 1))
        for hh in range(2):
            # contiguous partition-0 tiles only: the custom DVE/gpsimd ops
            # drop input partition offsets on HW, so stage the denominator
            # row at partition 0 with a standard copy first.
            dcp = work.tile([1, BW], F32, tag="dcp", name=f"dc{hp}{ib}{hh}")
            rden = work.tile([1, BW], F32, tag="rden", name=f"rd{hp}{ib}{hh}")
            bcst = work.tile([64, BW], F32, tag="bcs", name=f"bs{hp}{ib}{hh}")
            nc.vector.tensor_copy(out=dcp, in_=ctx_ps[hh][HD : HD + 1, :])
            nc.vector.reciprocal_approx_fast(out=rden, in_=dcp)
            nc.gpsimd.partition_broadcast(out_ap=bcst, in_ap=rden)
            dst = cxt_sb[64 * hh : 64 * hh + 64, hp, ib * BW : (ib + 1) * BW]
            nc.vector.tensor_tensor(
                out=dst, in0=ctx_ps[hh][0:HD, :], in1=bcst,
                op=mybir.AluOpType.mult)
            nc.vector.tensor_scalar_add(
                out=dst, in0=dst,
                scalar1=bv_sb[64 * hh : 64 * hh + 64, hp : hp + 1])

    def outproj(it):
        ps_o = [psum.tile([128, 512], F32, tag="pp", bufs=2,
                          name=f"po{it}_{eb}") for eb in range(2)]
        for ct in range(CT):
            for eb in range(2):
                nc.tensor.matmul(
                    ps_o[eb],
                    lhsT=cxt_sb[:, ct, it * 128 : (it + 1) * 128],
                    rhs=wo_sb[:, ct, eb * 512 : (eb + 1) * 512],
                    start=(ct == 0), stop=(ct == CT - 1))
        o_sb = work.tile([128, D], F16, tag="osb", name=f"os{it}")
        for eb in range(2):
            nc.vector.tensor_copy(out=o_sb[:, eb * 512 : (eb + 1) * 512],
                                  in_=ps_o[eb])
            # per-half DMA: overlaps the second copy, drains the tail sooner
            nc.sync.dma_start(out=out[it][:, eb * 512 : (eb + 1) * 512],
                              in_=o_sb[:, eb * 512 : (eb + 1) * 512])

    # software pipeline: outproj trails attn by one block so its PE work
    # and PSUM traffic never gate the next block's projections.
    for blk in range(NB):
        proj(blk)
        for hp in range(CT):
            attn(hp, blk)
        if blk > 0:
            for j in range(4):
                outproj(4 * (blk - 1) + j)
    for j in range(4):
        outproj(4 * (NB - 1) + j)

    for p in (psum, work, singles):
        p.release()


_CACHE = {}


def _compiled():
    if "nc" in _CACHE:
        return _CACHE["nc"]
    nc = bacc.Bacc("TRN2", target_bir_lowering=False, debug=False)
    io = {
        "xbT": nc.dram_tensor("xbT", [128, NB, DT, BW], F16,
                              kind="ExternalInput").ap(),
        "wq": nc.dram_tensor("wq", [128, DT, CPC], F16, kind="ExternalInput").ap(),
        "wk": nc.dram_tensor("wk", [128, DT, CPC], F16, kind="ExternalInput").ap(),
        "wv": nc.dram_tensor("wv", [128, DT, CPC], F16, kind="ExternalInput").ap(),
        "wo": nc.dram_tensor("wo", [128, CT, D], F16, kind="ExternalInput").ap(),
        "bqkv": nc.dram_tensor("bqkv", [128, 3 * CT], F32,
                               kind="ExternalInput").ap(),
        "out": nc.dram_tensor("out", [L, D], F16, kind="ExternalOutput").ap(),
    }
    with tile.TileContext(nc) as tc:
        build(tc, io)
    nc.compile()
    _CACHE["nc"] = nc
    return nc


def _pack(a, p=128):
    """[T*p, N] -> [p, T, N] (SBUF layout: partition-major)."""
    t = a.shape[0] // p
    return np.ascontiguousarray(
        a.reshape(t, p, a.shape[1]).transpose(1, 0, 2))


def _pack_x(xT):
    """[DT*128, NB*BW] -> [128, NB, DT, BW] (block-major SBUF layout)."""
    return np.ascontiguousarray(
        xT.reshape(DT, 128, NB, BW).transpose(1, 2, 0, 3))


def make_in_maps(x, W_qkv, b_qkv, W_lin):
    f16 = mybir.dt.np(F16)
    in_maps = []
    for c in range(N_CORES):
        b, g = divmod(c, 2)
        cs = slice(CPC * g, CPC * (g + 1))
        biases = [b_qkv[i * D + CPC * g : i * D + CPC * (g + 1)]
                  .astype(np.float32).reshape(CT, 128).T for i in range(3)]
        in_maps.append({
            "xbT": _pack_x(np.ascontiguousarray(x[b].T).astype(f16)),
            "wq": _pack(W_qkv[:, cs].astype(f16)),
            "wk": _pack(W_qkv[:, D + CPC * g : D + CPC * (g + 1)].astype(f16)),
            "wv": _pack(W_qkv[:, 2 * D + CPC * g : 2 * D + CPC * (g + 1)].astype(f16)),
            "wo": _pack(W_lin[cs, :].astype(f16)),
            "bqkv": np.ascontiguousarray(np.concatenate(biases, axis=1)),
        })
    return in_maps


def kernel(x, W_qkv, b_qkv, W_lin, b_lin, _trace=False):
    nc = _compiled()
    in_maps = make_in_maps(x, W_qkv, b_qkv, W_lin)
    res = run_bass_kernel_spmd(nc, in_maps, core_ids=list(range(N_CORES)),
                               trace=_trace)
    parts = [r["out"] for r in res.results]
    out = np.empty((B, L, D), dtype=np.float32)
    for b in range(B):
        out[b] = (parts[2 * b].astype(np.float32)
                  + parts[2 * b + 1].astype(np.float32)
                  + b_lin.astype(np.float32))
    if _trace:
        return out, res
    return out
